# revision 54
# baseline (speedup 1.0000x reference)
"""GCN layer kernel for Trainium2, distributed over 8 NeuronCores. (v2)

Math (matches the reference):
    support = X @ W                     # [N, D] GEMM (bf16 in, f32 psum)
    msgs    = support[edge_src] * edge_val[:, None]
    out     = segment_sum(msgs, edge_dst, N) + b

Distribution: 1D graph partition over destination rows. Core m owns dst rows
[m*RPC, (m+1)*RPC) and the edges that land there. Each core computes the full
`support` locally (bf16 X@W) into its own DRAM region, then gathers the
source rows it needs with `dma_gather` (the ~7.3ns/idx Q7 descriptor cost is
the kernel's critical path), builds scaled one-hot blocks on DVE, and
PE-matmuls them into per-window PSUM accumulated in an SBUF slab.

v2 changes vs v1:
  - Gather stream is packed per (chunk, window) cell WITHOUT rounding each
    cell to 128 slots; cell sizes are the max over the 8 cores (shared SPMD
    structure). 128-edge tiles may straddle two cells; each (tile, window)
    pass uses a sentinel-masked one-hot (dst_local=200 never matches iota)
    so foreign/empty slots contribute zero. Cuts gather descriptors ~9%.
  - Phase 1 runs in bf16 (Xt and W converted on host), halving PE time and
    Xt HBM traffic.
  - Uneven chunks: a small first chunk gets the gathers started early.
"""

import os
import numpy as np
import ml_dtypes

import concourse.bass as bass
import concourse.bacc as bacc
import concourse.mybir as mybir
import concourse.tile as tile
from concourse import bass_utils
from concourse.tile_rust import add_dep_helper

F32 = mybir.dt.float32
BF16 = mybir.dt.bfloat16
I16 = mybir.dt.int16

SENT = 200.0  # dst-local sentinel: never equals iota 0..127
NQ = 4        # SWDGE queues (ucode max 4): desc-gen parallelizes across
              # queues (HW-probed 8.65us -> 2.5us per 1024-idx gather)

# ---------------------------------------------------------------- config


class Cfg:
    def __init__(self, n_nodes, d, n_cores, chunk_bounds, gather_batch,
                 xw_block):
        self.n_nodes = n_nodes
        self.d = d                      # 256
        self.n_cores = n_cores
        self.rpc = n_nodes // n_cores   # dst rows per core
        self.cb = chunk_bounds          # chunk node boundaries (128-aligned
        self.n_chunks = len(chunk_bounds) - 1      # except the last)
        assert chunk_bounds[0] == 0 and chunk_bounds[-1] == n_nodes
        for a, b in zip(chunk_bounds, chunk_bounds[1:]):
            assert 0 < b - a <= 32640 and a % 128 == 0
        # p-major support layout: chunk c stores node nloc at padded row
        # (nloc%128)*ranks[c] + nloc//128 so phase-1 writes are contiguous
        # per partition. ranks[c] = padded chunk length / 128.
        self.ranks = [-(-(b - a) // 128)
                      for a, b in zip(chunk_bounds, chunk_bounds[1:])]
        self.pcb = [0]
        for r in self.ranks:
            self.pcb.append(self.pcb[-1] + 128 * r)
        assert all(128 * r <= 32768 for r in self.ranks)  # int16 idx
        self.gb = gather_batch          # max edges per dma_gather
        assert gather_batch % 128 == 0
        self.tpg = gather_batch // 128  # tiles per full gather
        self.nw = (self.rpc + 127) // 128   # dst windows per core
        self.xw_block = xw_block        # nodes per phase-1 block


# gather_batch=1024 (65 descriptors/ring) is HW-probed safe; 1408+ wedges.
# geometric chunk ramp: phase1(c+1) (~5.5ns/node wall) must hide inside
# phase2(c) (~12ns/node), so each chunk is at most ~2x the previous.
# Small last chunk keeps the post-gather tail short.
FULL = Cfg(n_nodes=100000, d=256, n_cores=8,
           chunk_bounds=[0, 4096, 12288, 28928, 61568, 94208, 100000],
           gather_batch=1024, xw_block=1024)


# ---------------------------------------------------------------- host prep


def _preprocess(cfg, edge_src, edge_dst, edge_val):
    """Bucket edges per (core, src-chunk, dst-window). Shared structure:
    cell slot count L[c,w] = max over cores (no 128 rounding); each chunk's
    stream is padded to a multiple of 128. Returns the shared structure and
    per-core packed index/dst/val arrays."""
    nw, nch = cfg.nw, cfg.n_chunks
    cbounds = np.asarray(cfg.cb)
    m_of = edge_dst // cfg.rpc
    per_core = []
    counts = np.zeros((cfg.n_cores, nch, nw), np.int64)
    for m in range(cfg.n_cores):
        sel = np.nonzero(m_of == m)[0]
        s = edge_src[sel]
        d = edge_dst[sel] - m * cfg.rpc
        v = edge_val[sel]
        c = (np.searchsorted(cbounds, s, side="right") - 1).astype(np.int64)
        w = (d >> 7).astype(np.int64)
        order = np.lexsort((s, w, c))
        s, d, v, c, w = s[order], d[order], v[order], c[order], w[order]
        counts[m] = np.bincount(c * nw + w, minlength=nch * nw).reshape(
            nch, nw)
        per_core.append((s, d, v))

    L = counts.max(axis=0)                       # [nch, nw] shared cell len
    # slot offset of each cell within its chunk; chunk stream len (pad 128)
    cell_off = np.zeros((nch, nw), np.int64)
    S = np.zeros(nch, np.int64)                  # padded chunk stream len
    for c in range(nch):
        off = np.concatenate([[0], np.cumsum(L[c])])
        cell_off[c] = off[:-1]
        S[c] = ((off[-1] + 127) // 128) * 128
    NI = int(S.sum())

    # shared pass list: execution order (c, w, tile) with psum start/stop
    passes = []                                  # (c, w, t, start, stop)
    np_per = 0
    for c in range(nch):
        for w in range(nw):
            a = int(cell_off[c, w])
            b = a + int(L[c, w])
            if b == a:
                continue
            ta, tb = a // 128, (b - 1) // 128
            for t in range(ta, tb + 1):
                passes.append((c, w, t, t == ta, t == tb))
                np_per += 1
    NP = len(passes)

    chunk_slot0 = np.concatenate([[0], np.cumsum(S)])
    core_arrays = []
    for m in range(cfg.n_cores):
        s, d, v = per_core[m]
        idx = np.zeros(NI, np.int16)
        dl = np.full(NP * 128, SENT, np.float32)  # per-pass dst locals
        vv = np.zeros(NP * 128, np.float32)
        # fill gather slots per cell
        e0 = 0
        for c in range(nch):
            for w in range(nw):
                k = int(counts[m, c, w])
                if k:
                    o = int(chunk_slot0[c] + cell_off[c, w])
                    nloc = s[e0:e0 + k] - cfg.cb[c]
                    idx[o:o + k] = ((nloc % 128) * cfg.ranks[c]
                                    + nloc // 128).astype(np.int16)
                    e0 += k
        # fill per-pass one-hot metadata
        e0 = 0
        cell_edge0 = {}
        for c in range(nch):
            for w in range(nw):
                k = int(counts[m, c, w])
                cell_edge0[(c, w)] = e0
                e0 += k
        for p, (c, w, t, _, _) in enumerate(passes):
            a = int(cell_off[c, w])
            b = a + int(L[c, w])
            k = int(counts[m, c, w])
            lo = max(a, t * 128)
            hi = min(a + k, b, (t + 1) * 128)
            if hi <= lo:
                continue
            eb = cell_edge0[(c, w)] + (lo - a)
            pp = p * 128 + (lo - t * 128)
            n = hi - lo
            dl[pp:pp + n] = (d[eb:eb + n] - w * 128).astype(np.float32)
            vv[pp:pp + n] = v[eb:eb + n]
        gidx = np.ascontiguousarray(
            np.tile(idx.reshape(NI // 16, 16).T, (8, 1)))      # [128, NI/16]
        dstl = np.ascontiguousarray(
            dl.reshape(NP, 128).T.astype(ml_dtypes.bfloat16))  # [128, NP]
        valt = np.ascontiguousarray(
            vv.reshape(NP, 128).T.astype(ml_dtypes.bfloat16))  # [128, NP]
        valf = np.ascontiguousarray(vv.reshape(NP, 128).T)     # [128, NP] f32
        core_arrays.append((gidx, dstl, valt, valf))
    return passes, S, NI, NP, core_arrays


# ---------------------------------------------------------------- device IR


def _build(tc, nc, cfg, passes, S, NP, ap):
    D = cfg.d
    n_full_w = cfg.rpc // 128
    tail_rows = cfg.rpc - n_full_w * 128
    sup = ap["support"]
    chunk_slot0 = [0]
    for c in range(cfg.n_chunks):
        chunk_slot0.append(chunk_slot0[-1] + int(S[c]))
    last_add = {}     # window -> its most recent slab-accumulate instr
    # support-write instructions per chunk: chunk c's gathers wait on these
    # through a single fan-in nop instead of an all-engine barrier drain
    sup_writes = [[] for _ in range(cfg.n_chunks)]

    def phase1_blocks(c, xtp, stp, psp, w0, w1):
        # p-major view of chunk c's support region: [128, ranks, D]
        sup_pm = sup[cfg.pcb[c]:cfg.pcb[c + 1], :].rearrange(
            "(p r) d -> p r d", p=128)
        n0 = cfg.cb[c]
        end = cfg.cb[c + 1]
        while n0 < end:
            yield
            nb = min(cfg.xw_block, end - n0)
            xt0 = xtp.tile([128, nb], BF16, tag="xt0")
            xt1 = xtp.tile([128, nb], BF16, tag="xt1")
            nc.sync.dma_start(xt0[:], ap["Xt"][0:128, n0:n0 + nb])
            nc.sync.dma_start(xt1[:], ap["Xt"][128:256, n0:n0 + nb])
            nj = (nb + 127) // 128
            stage = stp.tile([128, nj * D], BF16, tag="stage")
            for j in range(nj):
                if j:
                    yield   # fine-grained pacing: interleave feeder work
                m = min(128, nb - j * 128)
                ps = psp.tile([128, D], F32, tag="ps1")
                sl = slice(j * 128, j * 128 + m)
                nc.tensor.matmul(ps[0:m, :], xt0[:, sl], w0[:],
                                 start=True, stop=False)
                nc.tensor.matmul(ps[0:m, :], xt1[:, sl], w1[:],
                                 start=False, stop=True)
                nc.scalar.copy(stage[0:m, j * D:(j + 1) * D], ps[0:m, :])
            b0 = (n0 - cfg.cb[c]) // 128
            nfull = nb // 128
            if nfull:
                # contiguous per-partition write: 128 descs of nfull*512B
                wi = nc.sync.dma_start(
                    sup_pm[:, b0:b0 + nfull, :],
                    stage[:, 0:nfull * D].rearrange("p (j d) -> p j d", d=D))
                sup_writes[c].append(wi)
            if nb - nfull * 128:
                m = nb - nfull * 128
                wi = nc.sync.dma_start(
                    sup_pm[0:m, b0 + nfull:b0 + nfull + 1, :],
                    stage[0:m, nfull * D:(nfull + 1) * D].rearrange(
                        "p (o d) -> p o d", o=1))
                sup_writes[c].append(wi)
            n0 += nb

    GIG = 16                          # gathers per batched index load

    def load_gi(gip, c, bi):
        sc = int(S[c])
        w0c = bi * GIG * cfg.gb // 16
        wn = (min(sc, (bi + 1) * GIG * cfg.gb) // 16) - w0c
        git = gip.tile([128, wn], I16, tag="gi")
        col0 = chunk_slot0[c] // 16 + w0c
        # sync engine: pure-DMA stream, unlike scalar's (feeder
        # copies + one-hot scales would delay the load)
        nc.sync.dma_start(git[:], ap["gidx"][:, col0:col0 + wn])
        return git

    def phase2_chunk(c, pass0, pass1, pools, iota, dstlt, valt, valft,
                     slab, feeder=None, gi_pre=None):
        gbp, gip, ohp, ps2p = pools
        sup_c = sup[cfg.pcb[c]:cfg.pcb[c + 1], :]
        sc = int(S[c])
        n_g = (sc + cfg.gb - 1) // cfg.gb
        gbufs = [None] * n_g

        gi_tiles = [None] * ((n_g + GIG - 1) // GIG)
        if gi_pre is not None:
            gi_tiles[0] = gi_pre
        fed_due = 0.0
        fed_done = 0

        def ensure_gather(g):
            if gbufs[g] is None:
                bi = g // GIG
                if gi_tiles[bi] is None:
                    gi_tiles[bi] = load_gi(gip, c, bi)
                n = min(cfg.gb, sc - g * cfg.gb)
                gbuf = gbp.tile([128, n // 128, D], BF16, tag="gb")
                o = (g % GIG) * (cfg.gb // 16)
                nc.gpsimd.dma_gather(
                    gbuf[:], sup_c, gi_tiles[bi][:, o:o + n // 16],
                    num_idxs=n, num_idxs_reg=n, elem_size=D,
                    queue_num=g % NQ)
                gbufs[g] = gbuf
            return gbufs[g]

        # group passes of this chunk by window for psum accumulation
        p = pass0
        while p < pass1:
            c0, w, t0, st, _ = passes[p]
            assert c0 == c and st
            p_end = p
            while not passes[p_end][4]:
                p_end += 1
            k = p_end - p + 1                       # passes for window w
            oh = ohp.tile([128, k * 128], BF16, tag="oh")
            iota_b = iota[:].rearrange(
                "p (o f) -> p o f", o=1).broadcast_to([128, k, 128])
            dst_b = dstlt[:, p:p + k].rearrange(
                "p (f o) -> p f o", o=1).broadcast_to([128, k, 128])
            oh3 = oh[:].rearrange("p (o f) -> p o f", f=128)
            nc.vector.tensor_tensor(oh3, iota_b, dst_b,
                                    op=mybir.AluOpType.is_equal)
            # scale one-hot columns by edge values: split between DVE and
            # ACT so neither engine is the bottleneck
            k_dve = k // 2
            if k_dve:
                oh3a = oh[:, 0:k_dve * 128].rearrange(
                    "p (o f) -> p o f", f=128)
                val_b = valt[:, p:p + k_dve].rearrange(
                    "p (f o) -> p f o", o=1).broadcast_to([128, k_dve, 128])
                nc.vector.tensor_tensor(oh3a, oh3a, val_b,
                                        op=mybir.AluOpType.mult)
            for i in range(k_dve, k):
                nc.scalar.activation(
                    oh[:, i * 128:(i + 1) * 128],
                    oh[:, i * 128:(i + 1) * 128],
                    mybir.ActivationFunctionType.Copy,
                    scale=valft[:, p + i:p + i + 1])
            ps = ps2p.tile([128, D], F32, tag="ps2")
            for i in range(k):
                t = passes[p + i][2]
                gbuf = ensure_gather(t // cfg.tpg)
                nc.tensor.matmul(ps[:], oh[:, i * 128:(i + 1) * 128],
                                 gbuf[:, t % cfg.tpg, :],
                                 start=(i == 0), stop=(i == k - 1))
            sl = slab[:, w * D:(w + 1) * D]
            ai = nc.vector.tensor_tensor(sl, sl, ps[:],
                                         op=mybir.AluOpType.add)
            last_add[w] = ai
            if c == cfg.n_chunks - 1:
                # stream each finished window straight out; the explicit
                # edge pins the DMA after this window's final accumulation
                # (subtile RAW on the big slab tile alone proved unsafe)
                oi = nc.sync.dma_start(ap["out"][:, w * D:(w + 1) * D], sl)
                add_dep_helper(oi.ins, ai.ins, sync=True,
                               reason="window final before out DMA")
            p = p_end + 1
            if feeder is not None:
                fed_due += ypw
                while fed_done < fed_due:
                    next(feeder, None)
                    fed_done += 1
        if feeder is not None:
            for _ in feeder:
                pass

    # pass ranges per chunk
    chunk_pass0 = []
    cprev = -1
    for p, (c, _, _, _, _) in enumerate(passes):
        if c != cprev:
            chunk_pass0.append(p)
            cprev = c
    chunk_pass0.append(NP)

    with tc.tile_pool(name="const", bufs=1) as cp, \
         tc.tile_pool(name="slab", bufs=1) as slabp:
        w0 = cp.tile([128, D], BF16, tag="w0")
        w1 = cp.tile([128, D], BF16, tag="w1")
        nc.sync.dma_start(w0[:], ap["W"][0:128, :])
        nc.sync.dma_start(w1[:], ap["W"][128:256, :])
        bbt = cp.tile([128, D], F32, tag="bb")
        nc.sync.dma_start(bbt[:], ap["bb"][:, :])
        iota = cp.tile([128, 128], BF16, tag="iota")
        nc.gpsimd.iota(iota[:], pattern=[[1, 128]], base=0,
                       channel_multiplier=0,
                       allow_small_or_imprecise_dtypes=True)
        dstlt = cp.tile([128, NP], BF16, tag="dstl")
        nc.scalar.dma_start(dstlt[:], ap["dstl"][:, :])
        valt = cp.tile([128, NP], BF16, tag="val")
        nc.scalar.dma_start(valt[:], ap["val"][:, :])
        valft = cp.tile([128, NP], F32, tag="valf")
        nc.scalar.dma_start(valft[:], ap["valf"][:, :])

        slab = slabp.tile([128, cfg.nw * D], F32, tag="slab")
        nc.vector.tensor_copy(
            slab[:].rearrange("p (w d) -> p w d", d=D),
            bbt[:].rearrange("p (o d) -> p o d", o=1).broadcast_to(
                [128, cfg.nw, D]))

        with tc.tile_pool(name="xt", bufs=2) as xtp, \
             tc.tile_pool(name="stage", bufs=3) as stp, \
             tc.tile_pool(name="ps1", bufs=2, space="PSUM") as psp, \
             tc.tile_pool(name="gb", bufs=9) as gbp, \
             tc.tile_pool(name="gi", bufs=3) as gip, \
             tc.tile_pool(name="oh", bufs=4) as ohp, \
             tc.tile_pool(name="ps2", bufs=6, space="PSUM") as ps2p:
            pools = (gbp, gip, ohp, ps2p)
            gi0 = load_gi(gip, 0, 0)     # prefetch chunk-0 indices early
            for _ in phase1_blocks(0, xtp, stp, psp, w0, w1):
                pass
            for c in range(cfg.n_chunks):
                # compile-time fence keeps schedule order (no chunk c+1
                # instruction hoists before here), while the runtime wait
                # is only the gpsimd nop's edges on chunk c's support
                # writes -- no all-engine drain at chunk transitions.
                tc.no_sync_barrier()
                hb = nc.gpsimd.nop()
                for wi in sup_writes[c]:
                    add_dep_helper(hb.ins, wi.ins, sync=True,
                                   reason="support chunk ready")
                if c + 1 < cfg.n_chunks:
                    feeder = phase1_blocks(c + 1, xtp, stp, psp, w0, w1)
                    # fractional pacing: spread chunk c+1's psum tiles
                    # evenly over the windows (integer ceil front-loaded
                    # the feeder ~33% denser than needed), finishing ~8
                    # windows early so the writes drain before handoff
                    ypw = cfg.ranks[c + 1] / max(1.0, cfg.nw - 8.0)
                else:
                    feeder = None
                    ypw = 0.0
                phase2_chunk(c, chunk_pass0[c], chunk_pass0[c + 1],
                             pools, iota, dstlt, valt, valft, slab,
                             feeder, gi_pre=(gi0 if c == 0 else None))

        # ---------------- output (p-major; host unscrambles) -----------
        # windows with edges in the last chunk were streamed out above;
        # cover any window absent from the last chunk's pass list.
        last_w = {w for (c, w, _, _, _) in passes if c == cfg.n_chunks - 1}
        for w in range(cfg.nw):
            if w not in last_w:
                oi = nc.sync.dma_start(ap["out"][:, w * D:(w + 1) * D],
                                       slab[:, w * D:(w + 1) * D])
                if w in last_add:
                    add_dep_helper(oi.ins, last_add[w].ins, sync=True,
                                   reason="window final before out DMA")


def build_program(cfg, passes, S, NI, NP, debug=False):
    nc = bacc.Bacc("TRN2", target_bir_lowering=False, debug=debug,
                   enable_asserts=False, num_devices=cfg.n_cores,
                   num_swdge_queues=NQ)
    ap = {
        "Xt": nc.dram_tensor("Xt", [cfg.d, cfg.n_nodes], BF16,
                             kind="ExternalInput").ap(),
        "W": nc.dram_tensor("W", [cfg.d, cfg.d], BF16,
                            kind="ExternalInput").ap(),
        "bb": nc.dram_tensor("bb", [128, cfg.d], F32,
                             kind="ExternalInput").ap(),
        "gidx": nc.dram_tensor("gidx", [128, NI // 16], I16,
                               kind="ExternalInput").ap(),
        "dstl": nc.dram_tensor("dstl", [128, NP], BF16,
                               kind="ExternalInput").ap(),
        "val": nc.dram_tensor("val", [128, NP], BF16,
                              kind="ExternalInput").ap(),
        "valf": nc.dram_tensor("valf", [128, NP], F32,
                               kind="ExternalInput").ap(),
        "out": nc.dram_tensor("out", [128, cfg.nw * cfg.d], F32,
                              kind="ExternalOutput").ap(),
        "support": nc.dram_tensor("support", [cfg.pcb[-1], cfg.d], BF16,
                                  kind="Internal").ap(),
    }
    with tile.TileContext(nc) as tc:
        _build(tc, nc, cfg, passes, S, NP, ap)
    nc.compile()
    return nc


# ---------------------------------------------------------------- entry


last_run_info = {}


def kernel(X, edge_src, edge_dst, edge_val, W, b):
    cfg = FULL
    X = np.asarray(X, np.float32)
    W = np.asarray(W, np.float32)
    b = np.asarray(b, np.float32)
    edge_src = np.asarray(edge_src, np.int32)
    edge_dst = np.asarray(edge_dst, np.int32)
    edge_val = np.asarray(edge_val, np.float32)

    passes, S, NI, NP, core_arrays = _preprocess(cfg, edge_src, edge_dst,
                                                 edge_val)
    nc = build_program(cfg, passes, S, NI, NP)

    Xt = np.ascontiguousarray(X.T.astype(ml_dtypes.bfloat16))
    Wb = np.ascontiguousarray(W.astype(ml_dtypes.bfloat16))
    bb = np.ascontiguousarray(np.broadcast_to(b, (128, cfg.d)))
    in_maps = []
    for m in range(cfg.n_cores):
        gidx, dstl, valt, valf = core_arrays[m]
        in_maps.append({"Xt": Xt, "W": Wb, "bb": bb, "gidx": gidx,
                        "dstl": dstl, "val": valt, "valf": valf})

    trace = bool(int(os.environ.get("GCN_TRACE", "0")))
    res = bass_utils.run_bass_kernel_spmd(
        nc, in_maps, core_ids=list(range(cfg.n_cores)), trace=trace)
    last_run_info.clear()
    last_run_info.update(exec_time_ns=res.exec_time_ns,
                         profile_json=res.profile_json)

    parts = []
    for m in range(cfg.n_cores):
        arr = np.asarray(res.results[m]["out"]).reshape(128, cfg.nw, cfg.d)
        parts.append(arr.transpose(1, 0, 2).reshape(cfg.nw * 128,
                                                    cfg.d)[:cfg.rpc])
    return np.concatenate(parts, axis=0)

